# revision 8
# baseline (speedup 1.0000x reference)
"""AceStepLyricEncoder forward on 8 Trainium2 NeuronCores.

Sharding: DP2 (batch) x TP4 (Megatron): cores 0-3 handle batch 0, cores 4-7
batch 1. Within a group: q/k/v/o projections sharded over heads, MLP over
intermediate dim; RMSNorm scales are per-token and either commute through
the projections (folded post-hoc) or are scale-invariant (q/k norms).

Layout: activations kept feature-major ("T-layout", [feature, token]) so
every matmul contracts over the partition axis with zero transposes.
Weights are pre-transposed/pre-sharded/pre-folded (ln1/ln2 into W) on the
host and shipped bf16. Residual stream stays fp32 in DRAM; all matmuls run
bf16; per-layer partial sums AllReduce in bf16 over 4-core groups.

The layer body is software-pipelined over the two token halves so each
AllReduce overlaps compute:
  attn(A)+O(A) -> AR_att(A) || attn(B)+O(B) -> AR_att(B)
  || resid(A)+s2(A)+gu(A)+down(A) -> AR_mlp(A) || resid(B)+...(B) -> AR_mlp(B)
  || resid2(A) + next layer's s1/V/K/Q(A)  || resid2(B) + s1/V/K/Q(B)
"""
import numpy as np
import ml_dtypes

import concourse.bass as bass
import concourse.mybir as mybir
import concourse.tile as tile
from concourse.tile_rust import add_dep_helper
from concourse import bacc
from concourse.bass_utils import run_bass_kernel_spmd

# model dims (hardcoded per problem spec)
L = 8
H = 16
HKV = 8
D = 128
HID = 2048
INT = 6144
TIN = 1024
WIN = 128
EPS = 1e-6
THETA = 1000000.0
B = 2
S = 1024

P = 128
TP = 4                 # tensor-parallel degree (cores per batch group)
NQ = H // TP           # 4 q heads per core
NKV = HKV // TP        # 2 kv heads per core
IS = INT // TP         # 1536 intermediate per core
FC = HID // P          # 16 feature chunks
IC = TIN // P          # 8 input-dim chunks
OC = IS // P           # 12 intermediate chunks
TC = S // P            # 8 token chunks
NH = 2                 # token halves (AR chunking)
HW = S // NH           # 512 tokens per half

BF = mybir.dt.bfloat16
F32 = mybir.dt.float32

GROUPS = [[0, 1, 2, 3], [4, 5, 6, 7]]
SLIDING = [l % 2 == 0 for l in range(L)]  # ['sliding', 'full'] * 4


def build_program(n_layers=L, repeat=1, ar_f32=False, no_coll=False):
    ARD = F32 if ar_f32 else BF
    nc = bacc.Bacc("TRN2", target_bir_lowering=False, debug=False, num_devices=8)

    # ---- DRAM I/O ----
    xb = nc.dram_tensor("xb", [TIN, S], BF, kind="ExternalInput").ap()
    wp = nc.dram_tensor("wp", [TIN, HID], BF, kind="ExternalInput").ap()
    wq = nc.dram_tensor("wq", [n_layers, HID, NQ * D], BF, kind="ExternalInput").ap()
    wk = nc.dram_tensor("wk", [n_layers, HID, NKV * D], BF, kind="ExternalInput").ap()
    wv = nc.dram_tensor("wv", [n_layers, HID, NKV * D], BF, kind="ExternalInput").ap()
    wo = nc.dram_tensor("wo", [n_layers, NQ * D, HID], BF, kind="ExternalInput").ap()
    wg = nc.dram_tensor("wg", [n_layers, HID, IS], BF, kind="ExternalInput").ap()
    wu = nc.dram_tensor("wu", [n_layers, HID, IS], BF, kind="ExternalInput").ap()
    wd = nc.dram_tensor("wd", [n_layers, IS, HID], BF, kind="ExternalInput").ap()
    cq = nc.dram_tensor("cq", [n_layers, D, S], BF, kind="ExternalInput").ap()
    sq = nc.dram_tensor("sq", [n_layers, D, S], BF, kind="ExternalInput").ap()
    ck = nc.dram_tensor("ck", [n_layers, D, S], BF, kind="ExternalInput").ap()
    sk = nc.dram_tensor("sk", [n_layers, D, S], BF, kind="ExternalInput").ap()
    nw = nc.dram_tensor("nw", [HID, 1], F32, kind="ExternalInput").ap()
    mge = nc.dram_tensor("mge", [P, P], BF, kind="ExternalInput").ap()
    mle = nc.dram_tensor("mle", [P, P], BF, kind="ExternalInput").ap()
    out = nc.dram_tensor("out", [HID, S], F32, kind="ExternalOutput").ap()

    with tile.TileContext(nc) as tc:
        with (
            tc.tile_pool(name="persist", bufs=1) as pp,
            tc.tile_pool(name="work", bufs=1) as pwk,
            tc.tile_pool(name="psum", bufs=1, space="PSUM") as pps,
            tc.tile_pool(name="dram", bufs=1, space="DRAM") as pdr,
        ):
            # ---- persistent SBUF state: bf16 copy of residual stream ----
            hb = [pp.tile([P, S], BF, name=f"hb{f}") for f in range(FC)]
            # ---- persistent DRAM state: fp32 residual stream ----
            hd = [pdr.tile([P, S], F32, name=f"hd{f}", bufs=1) for f in range(FC)]

            ones_b = pp.tile([P, P], BF, name="ones_b")
            nc.vector.memset(ones_b[:], 1.0)
            ones_f = pp.tile([1, P], F32, name="ones_f")
            nc.vector.memset(ones_f[:], 1.0)
            m_ge = pp.tile([P, P], BF, name="m_ge")
            nc.sync.dma_start(m_ge[:], mge)
            m_le = pp.tile([P, P], BF, name="m_le")
            nc.sync.dma_start(m_le[:], mle)
            nw_sb = pp.tile([P, FC], F32, name="nw_sb")
            nc.sync.dma_start(nw_sb[:], nw.rearrange("(f p) one -> p (f one)", p=P))
            eps_c = pp.tile([P, 1], F32, name="eps_c")
            nc.vector.memset(eps_c[:], EPS)

            def t_new(shape, dt, tag, bufs):
                return pwk.tile(shape, dt, tag=tag, name=tag, bufs=bufs)

            def psum(shape, tag, bufs):
                return pps.tile(shape, F32, tag=tag, name=tag, bufs=bufs)

            # weight strip loaders (blocked streaming; each strip's live-set
            # stays below its tag's bufs)
            def load_strips(dram_ap, l, kchunks, cols, tag, bufs, eng=None):
                eng = eng or nc.sync
                ts_ = []
                for i in range(kchunks):
                    t = t_new([P, cols.stop - cols.start], BF, tag, bufs)
                    eng.dma_start(t[:], dram_ap[l, i * P:(i + 1) * P, cols])
                    ts_.append(t)
                return ts_

            # ---------------- input projection ----------------
            # x resident in SBUF once (2 MB, reuses the attention "kb"/"ob"
            # shaped pools is not possible -> stream via exp-sized tag), wp
            # streamed in 4 blocks
            for nh in range(NH):
                xs_sb = []
                for i in range(IC):
                    t = t_new([P, HW], BF, "xt", 8)
                    nc.sync.dma_start(t[:], xb[i * P:(i + 1) * P, nh * HW:(nh + 1) * HW])
                    xs_sb.append(t)
                for fb in range(4):                  # blocks of 4 f-chunks
                    wps = []
                    for i in range(IC):
                        t = t_new([P, 512], BF, "w512", 14)
                        nc.sync.dma_start(t[:], wp[i * P:(i + 1) * P, fb * 512:(fb + 1) * 512])
                        wps.append(t)
                    for fo in range(4):
                        f = fb * 4 + fo
                        ps = psum([P, HW], "pm", 2)
                        for i in range(IC):
                            nc.tensor.matmul(ps[:], wps[i][:, fo * P:(fo + 1) * P],
                                             xs_sb[i][:],
                                             start=(i == 0), stop=(i == IC - 1))
                        ho = t_new([P, HW], F32, "hio", 3)
                        nc.vector.tensor_copy(ho[:], ps[:])
                        nc.sync.dma_start(hd[f][:, nh * HW:(nh + 1) * HW], ho[:])
                        nc.vector.tensor_copy(hb[f][:, nh * HW:(nh + 1) * HW], ho[:])

            def qk_head(ps, cq_t, sq_t, half, out_ap):
                """rms-normalize (scale-invariant, eps approx) + rope; psum ->
                bf16 out_ap [P, HW]."""
                sqt = t_new([P, HW], BF, "sq", 2)
                nc.scalar.activation(sqt[:], ps[:], mybir.ActivationFunctionType.Square)
                ss = psum([P, HW], "psm", 2)
                nc.tensor.matmul(ss[:], ones_b[:], sqt[:], start=True, stop=True)
                sr = t_new([P, HW], F32, "f32t", 3)
                nc.scalar.activation(sr[:], ss[:], mybir.ActivationFunctionType.Sqrt,
                                     scale=1.0 / D, bias=eps_c[:, :])
                rs = t_new([P, HW], F32, "rsq", 2)
                nc.vector.reciprocal(rs[:], sr[:])
                qsb = t_new([P, HW], F32, "qsb", 2)
                nc.vector.tensor_copy(qsb[:], ps[:])
                qsh = t_new([P, HW], F32, "qsh", 1)
                nc.sync.dma_start(qsh[0:64, :], qsb[64:128, :])
                nc.sync.dma_start(qsh[64:128, :], qsb[0:64, :])
                hs = slice(half * HW, (half + 1) * HW)
                t1 = t_new([P, HW], F32, "tmp", 2)
                nc.vector.tensor_mul(t1[:], qsb[:], cq_t[:, hs])
                t2 = t_new([P, HW], F32, "tmp2", 1)
                nc.vector.tensor_mul(t2[:], qsh[:], sq_t[:, hs])
                nc.vector.tensor_add(t1[:], t1[:], t2[:])
                return nc.vector.tensor_mul(out_ap, t1[:], rs[:])

            # ---- persistent per-layer tiles (single-buffered: the pipeline
            # never overlaps two layers' uses of these) ----
            rope_bufs = 1

            def load_rope(l):
                cq_t = t_new([P, S], BF, "tcq", rope_bufs)
                nc.sync.dma_start(cq_t[:], cq[l])
                sq_t = t_new([P, S], BF, "tsq", rope_bufs)
                nc.sync.dma_start(sq_t[:], sq[l])
                ck_t = t_new([P, S], BF, "tck", rope_bufs)
                nc.sync.dma_start(ck_t[:], ck[l])
                sk_t = t_new([P, S], BF, "tsk", rope_bufs)
                nc.sync.dma_start(sk_t[:], sk[l])
                return cq_t, sq_t, ck_t, sk_t

            def s1_half(nh, vsc, s1_src):
                """sumsq of residual over features for half nh -> vsc[:, 4nh:4nh+4]"""
                hs = slice(nh * HW, (nh + 1) * HW)
                s1p = psum([1, HW], "psm", 2)
                for f in range(FC):
                    sqt = t_new([P, HW], BF, "sq", 2)
                    nc.scalar.activation(sqt[:], s1_src[f][:, hs],
                                         mybir.ActivationFunctionType.Square)
                    nc.tensor.matmul(s1p[:], ones_b[:, 0:1], sqt[:],
                                     start=(f == 0), stop=(f == FC - 1))
                s1r = t_new([1, HW], F32, "s1r", 2)
                nc.scalar.activation(s1r[:], s1p[:],
                                     mybir.ActivationFunctionType.Sqrt,
                                     scale=1.0 / HID, bias=eps_c[0:1, :])
                nc.vector.reciprocal(s1r[:], s1r[:])
                s1d = pdr.tile([1, HW], F32, tag="s1d", name="s1d", bufs=2)
                nc.sync.dma_start(s1d[:], s1r[:])
                nc.sync.dma_start(vsc[:, nh * (TC // NH):(nh + 1) * (TC // NH)],
                                  s1d[0, :].rearrange("(c p) -> p c", p=P))

            def v_half(nh, wv_s, vb, vsc):
                """V projection for token chunks of half nh (token-major)."""
                for c in range(nh * (TC // NH), (nh + 1) * (TC // NH)):
                    ps = psum([P, NKV * D], "pm", 2)
                    for i in range(FC):
                        nc.tensor.matmul(ps[:], hb[i][:, c * P:(c + 1) * P], wv_s[i][:],
                                         start=(i == 0), stop=(i == FC - 1))
                    nc.vector.tensor_scalar_mul(vb[:, c, :], ps[:], vsc[:, c:c + 1])

            def k_half(nh, wk_s, kb, ck_t, sk_t):
                for hk in range(NKV):
                    ps = psum([P, HW], "pm", 2)
                    for i in range(FC):
                        nc.tensor.matmul(ps[:], wk_s[i][:, hk * P:(hk + 1) * P],
                                         hb[i][:, nh * HW:(nh + 1) * HW],
                                         start=(i == 0), stop=(i == FC - 1))
                    last = qk_head(ps, ck_t, sk_t, nh, kb[:, hk, nh * HW:(nh + 1) * HW])
                return last

            def q_half(nh, l, qbh, cq_t, sq_t):
                for hp_ in range(NQ // 2):
                    wq_s = load_strips(wq, l, FC, slice(hp_ * 2 * D, (hp_ + 1) * 2 * D),
                                       "wqgu", 36)
                    for hq in (hp_ * 2, hp_ * 2 + 1):
                        ho_ = hq - hp_ * 2
                        ps = psum([P, HW], "pm", 2)
                        for i in range(FC):
                            nc.tensor.matmul(ps[:], wq_s[i][:, ho_ * P:(ho_ + 1) * P],
                                             hb[i][:, nh * HW:(nh + 1) * HW],
                                             start=(i == 0), stop=(i == FC - 1))
                        last = qk_head(ps, cq_t, sq_t, nh, qbh[hq][:, nh * HW:(nh + 1) * HW])
                return last

            def attn_half(nh, sliding, kb, qbh, vb, ob):
                for hq in range(NQ):
                    kv = hq // 2
                    if not sliding:
                        av = psum([P, HW], "pav", 2)
                        dacc = t_new([P, HW], BF, "dacc", 2)
                        for kt in range(TC):
                            sc = psum([P, HW], "psc", 2)
                            nc.tensor.matmul(sc[:], kb[:, kv, kt * P:(kt + 1) * P],
                                             qbh[hq][:, nh * HW:(nh + 1) * HW],
                                             start=True, stop=True)
                            ex = t_new([P, HW], BF, "exp", 3)
                            nc.scalar.activation(ex[:], sc[:], mybir.ActivationFunctionType.Exp)
                            nc.tensor.matmul(av[:], vb[:, kt, kv * D:(kv + 1) * D], ex[:],
                                             start=(kt == 0), stop=(kt == TC - 1))
                            if kt == 0:
                                nc.vector.tensor_copy(dacc[:], ex[:])
                            else:
                                nc.vector.tensor_add(dacc[:], dacc[:], ex[:])
                        dn = psum([1, HW], "psm", 2)
                        nc.tensor.matmul(dn[:], ones_b[:, 0:1], dacc[:], start=True, stop=True)
                        dr = t_new([1, HW], F32, "drow", 2)
                        nc.vector.reciprocal(dr[:], dn[:])
                        bc = psum([P, HW], "psm", 2)
                        nc.tensor.matmul(bc[:], ones_f[:], dr[:], start=True, stop=True)
                        bcs = t_new([P, HW], F32, "f32t", 3)
                        nc.vector.tensor_copy(bcs[:], bc[:])
                        nc.vector.tensor_mul(ob[:, hq, nh * HW:(nh + 1) * HW], av[:], bcs[:])
                    else:
                        for qc in range(nh * (TC // NH), (nh + 1) * (TC // NH)):
                            kts = [k for k in (qc - 1, qc, qc + 1) if 0 <= k < TC]
                            av = psum([P, P], "pav", 2)
                            dacc = t_new([P, P], BF, "dacc", 2)
                            for j, kt in enumerate(kts):
                                sc = psum([P, P], "psc", 2)
                                nc.tensor.matmul(sc[:], kb[:, kv, kt * P:(kt + 1) * P],
                                                 qbh[hq][:, qc * P:(qc + 1) * P],
                                                 start=True, stop=True)
                                ex = t_new([P, P], BF, "exp", 3)
                                nc.scalar.activation(ex[:], sc[:], mybir.ActivationFunctionType.Exp)
                                if kt == qc - 1:
                                    nc.vector.tensor_mul(ex[:], ex[:], m_ge[:])
                                elif kt == qc + 1:
                                    nc.vector.tensor_mul(ex[:], ex[:], m_le[:])
                                nc.tensor.matmul(av[:], vb[:, kt, kv * D:(kv + 1) * D], ex[:],
                                                 start=(j == 0), stop=(j == len(kts) - 1))
                                if j == 0:
                                    nc.vector.tensor_copy(dacc[:], ex[:])
                                else:
                                    nc.vector.tensor_add(dacc[:], dacc[:], ex[:])
                            dn = psum([1, P], "psm", 2)
                            nc.tensor.matmul(dn[:], ones_b[:, 0:1], dacc[:], start=True, stop=True)
                            dr = t_new([1, P], F32, "drow", 2)
                            nc.vector.reciprocal(dr[:], dn[:])
                            bc = psum([P, P], "psm", 2)
                            nc.tensor.matmul(bc[:], ones_f[:], dr[:], start=True, stop=True)
                            bcs = t_new([P, P], F32, "f32t", 3)
                            nc.vector.tensor_copy(bcs[:], bc[:])
                            nc.vector.tensor_mul(ob[:, hq, qc * P:(qc + 1) * P], av[:], bcs[:])

            def oproj_half(nh, l, ob):
                """O projection for half nh -> DRAM AR input; returns AR out."""
                bi = pdr.tile([HID, HW], ARD, tag="arin", name="arin", bufs=4)
                bo = pdr.tile([HID, HW], ARD, tag="arout", name="arout", bufs=4)
                for fb in range(4):
                    wo_s = []
                    for od in range(NQ):
                        t = t_new([P, 512], BF, "w512", 14)
                        nc.scalar.dma_start(t[:], wo[l, od * P:(od + 1) * P,
                                                     fb * 512:(fb + 1) * 512])
                        wo_s.append(t)
                    for fo in range(4):
                        f = fb * 4 + fo
                        ps = psum([P, HW], "pm", 2)
                        for od in range(NQ):
                            nc.tensor.matmul(ps[:], wo_s[od][:, fo * P:(fo + 1) * P],
                                             ob[:, od, nh * HW:(nh + 1) * HW],
                                             start=(od == 0), stop=(od == NQ - 1))
                        st_ = t_new([P, HW], ARD, "st", 3)
                        nc.vector.tensor_copy(st_[:], ps[:])
                        last = nc.scalar.dma_start(bi[f * P:(f + 1) * P, :], st_[:])
                if no_coll:
                    return bi, last
                nc.gpsimd.collective_compute(
                    "AllReduce", mybir.AluOpType.add, replica_groups=GROUPS,
                    ins=[bi.opt()], outs=[bo.opt()])
                return bo, last

            def _gate(inst, gate):
                # order-only edge: keep AR-consuming chains from being
                # scheduled ahead of the phase that should overlap the AR
                if gate is not None:
                    add_dep_helper(inst.ins, gate.ins, sync=False,
                                   reason="pipeline phase gate")

            def resid_s2_half(nh, ar_buf, s2b, gate=None):
                """residual add from AR + hb refresh + rms denom for half nh."""
                hs = slice(nh * HW, (nh + 1) * HW)
                for f in range(FC):
                    ld = t_new([P, HW], ARD, "ld", 3)
                    _gate(nc.sync.dma_start(ld[:], ar_buf[f * P:(f + 1) * P, :]), gate)
                    ho = t_new([P, HW], F32, "hio", 3)
                    nc.sync.dma_start(ho[:], hd[f][:, hs])
                    nc.vector.tensor_add(ho[:], ho[:], ld[:])
                    nc.scalar.dma_start(hd[f][:, hs], ho[:])
                    nc.vector.tensor_copy(hb[f][:, hs], ho[:])
                ss = psum([P, HW], "psm", 2)
                for f in range(FC):
                    sqt = t_new([P, HW], BF, "sq", 2)
                    nc.scalar.activation(sqt[:], hb[f][:, hs],
                                         mybir.ActivationFunctionType.Square)
                    nc.tensor.matmul(ss[:], ones_b[:], sqt[:],
                                     start=(f == 0), stop=(f == FC - 1))
                sr = t_new([P, HW], F32, "f32t", 3)
                nc.scalar.activation(sr[:], ss[:], mybir.ActivationFunctionType.Sqrt,
                                     scale=1.0 / HID, bias=eps_c[:, :])
                rs2 = t_new([P, HW], F32, "f32t", 3)
                nc.vector.reciprocal(rs2[:], sr[:])
                nc.vector.tensor_copy(s2b[:, hs], rs2[:])

            def gu_half(nh, l, gu, s2b):
                hs = slice(nh * HW, (nh + 1) * HW)
                for blk in range(OC // 2):        # 6 blocks of 2 o-chunks
                    ocols = slice(blk * 2 * P, (blk + 1) * 2 * P)
                    # alternate strip tags so block k+1 prefetches during
                    # block k's matmuls ("wkv" slots are idle during MLP)
                    tg, tb = ("wqgu", 36) if blk % 2 == 0 else ("wkv", 34)
                    wg_s = load_strips(wg, l, FC, ocols, tg, tb, eng=nc.sync)
                    wu_s = load_strips(wu, l, FC, ocols, tg, tb, eng=nc.scalar)
                    for oo in range(2):
                        o = blk * 2 + oo
                        pg = psum([P, HW], "pm", 2)
                        for i in range(FC):
                            nc.tensor.matmul(pg[:], wg_s[i][:, oo * P:(oo + 1) * P],
                                             hb[i][:, hs], start=(i == 0), stop=(i == FC - 1))
                        gsc = t_new([P, HW], F32, "tmp", 2)
                        nc.vector.tensor_mul(gsc[:], pg[:], s2b[:, hs])
                        gg = t_new([P, HW], BF, "gg", 2)
                        nc.scalar.activation(gg[:], gsc[:], mybir.ActivationFunctionType.Silu)
                        pu = psum([P, HW], "pm", 2)
                        for i in range(FC):
                            nc.tensor.matmul(pu[:], wu_s[i][:, oo * P:(oo + 1) * P],
                                             hb[i][:, hs], start=(i == 0), stop=(i == FC - 1))
                        uu = t_new([P, HW], BF, "uu", 2)
                        nc.vector.tensor_mul(uu[:], pu[:], s2b[:, hs])
                        nc.vector.tensor_mul(gu[:, o, hs], gg[:], uu[:])

            def down_half(nh, l, gu):
                hs = slice(nh * HW, (nh + 1) * HW)
                bi = pdr.tile([HID, HW], ARD, tag="arin", name="arin", bufs=4)
                bo = pdr.tile([HID, HW], ARD, tag="arout", name="arout", bufs=4)
                for fb in range(4):
                    wd_s = []
                    for o in range(OC):
                        # alternate tags ("xt" idle after input proj) so the
                        # next fb block prefetches during this one's matmuls
                        if fb % 2 == 0:
                            t = t_new([P, 512], BF, "w512", 14)
                        else:
                            t = t_new([P, 512], BF, "xt", 8) if o < 8 else                                 t_new([P, 512], BF, "w512", 14)
                        nc.scalar.dma_start(t[:], wd[l, o * P:(o + 1) * P,
                                                     fb * 512:(fb + 1) * 512])
                        wd_s.append(t)
                    for fo in range(4):
                        f = fb * 4 + fo
                        ps = psum([P, HW], "pm", 2)
                        for o in range(OC):
                            nc.tensor.matmul(ps[:], wd_s[o][:, fo * P:(fo + 1) * P],
                                             gu[:, o, hs], start=(o == 0), stop=(o == OC - 1))
                        st_ = t_new([P, HW], ARD, "st", 3)
                        nc.vector.tensor_copy(st_[:], ps[:])
                        last = nc.scalar.dma_start(bi[f * P:(f + 1) * P, :], st_[:])
                if no_coll:
                    return bi, last
                nc.gpsimd.collective_compute(
                    "AllReduce", mybir.AluOpType.add, replica_groups=GROUPS,
                    ins=[bi.opt()], outs=[bo.opt()])
                return bo, last

            def resid2_half(nh, ar_buf, fin_ss, gate=None):
                hs = slice(nh * HW, (nh + 1) * HW)
                if fin_ss is not None:
                    fss = psum([P, HW], "psm", 2)
                    fin_ss.append(fss)
                for f in range(FC):
                    ld = t_new([P, HW], ARD, "ld", 3)
                    _gate(nc.sync.dma_start(ld[:], ar_buf[f * P:(f + 1) * P, :]), gate)
                    ho = t_new([P, HW], F32, "hio", 3)
                    nc.sync.dma_start(ho[:], hd[f][:, hs])
                    nc.vector.tensor_add(ho[:], ho[:], ld[:])
                    nc.scalar.dma_start(hd[f][:, hs], ho[:])
                    nc.vector.tensor_copy(hb[f][:, hs], ho[:])
                    if fin_ss is not None:
                        sqt = t_new([P, HW], BF, "sq", 2)
                        nc.scalar.activation(sqt[:], ho[:],
                                             mybir.ActivationFunctionType.Square)
                        nc.tensor.matmul(fin_ss[-1][:], ones_b[:], sqt[:],
                                         start=(f == 0), stop=(f == FC - 1))

            def qkv_half(nh, l, vsc, vb, kb, qbh, ropes, wv_s, wk_s):
                """s1 + V + K + Q for half nh of layer l (after resid2(nh))."""
                cq_t, sq_t, ck_t, sk_t = ropes
                s1_half(nh, vsc, hb)
                v_half(nh, wv_s, vb, vsc)
                k_half(nh, wk_s, kb, ck_t, sk_t)
                return q_half(nh, l, qbh, cq_t, sq_t)

            def load_wvk(l):
                wv_s = load_strips(wv, l, FC, slice(0, NKV * D), "wkv", 34)
                wk_s = load_strips(wk, l, FC, slice(0, NKV * D), "wkv", 34)
                return wv_s, wk_s

            def layer_state():
                vsc = t_new([P, TC], F32, "vsc", 2)
                vb = t_new([P, TC, NKV * D], BF, "vb", 1)
                kb = t_new([P, NKV, S], BF, "kb", 1)
                qbh = [t_new([P, S], BF, f"qb{_h}", 1) for _h in range(NQ)]
                return vsc, vb, kb, qbh

            # ---------------- layers (software-pipelined) ----------------
            fin_ss = []   # final-norm sumsq accumulators, fused into last addback
            total_layers = repeat * n_layers

            # prologue: QKV for layer 0
            ropes = load_rope(0)
            wvk = load_wvk(0)
            st = layer_state()
            vsc, vb, kb, qbh = st
            for nh in range(NH):
                qkv_half(nh, 0, vsc, vb, kb, qbh, ropes, wvk[0], wvk[1])

            for gl in range(total_layers):
                l = gl % n_layers
                nl = (gl + 1) % n_layers
                last_ = gl == total_layers - 1
                sliding = SLIDING[l]
                vsc, vb, kb, qbh = st

                # ---- attention + O-projection + AR per half ----
                ob = t_new([P, NQ, S], BF, "ob", 1)
                ar_att = []
                for nh in range(NH):
                    attn_half(nh, sliding, kb, qbh, vb, ob)
                    ar_att.append(oproj_half(nh, l, ob))

                # ---- MLP per half (resid -> s2 -> gu -> down -> AR) ----
                s2b = t_new([P, S], BF, "s2b", 1)
                gu = t_new([P, OC, S], BF, "gu", 1)
                ar_mlp = []
                gates = [ar_att[1][1], None]
                for nh in range(NH):
                    resid_s2_half(nh, ar_att[nh][0], s2b, gate=gates[nh])
                    gu_half(nh, l, gu, s2b)
                    ar_mlp.append(down_half(nh, l, gu))
                    gates[1] = ar_mlp[0][1]

                # ---- residual 2 + next layer's QKV per half ----
                if not last_:
                    ropes = load_rope(nl)
                    wvk = load_wvk(nl)
                    st = layer_state()
                gate = ar_mlp[1][1]
                for nh in range(NH):
                    resid2_half(nh, ar_mlp[nh][0], fin_ss if last_ else None,
                                gate=gate)
                    if not last_:
                        gate = qkv_half(nh, nl, st[0], st[1], st[2], st[3], ropes,
                                        wvk[0], wvk[1])

            # ---------------- final norm (fp32 h from DRAM) ----------------
            for nh in range(NH):
                hs = slice(nh * HW, (nh + 1) * HW)
                ss = fin_ss[nh]
                sr = t_new([P, HW], F32, "f32t", 3)
                nc.scalar.activation(sr[:], ss[:], mybir.ActivationFunctionType.Sqrt,
                                     scale=1.0 / HID, bias=eps_c[:, :])
                rs = t_new([P, HW], F32, "rsf", 2)
                nc.vector.reciprocal(rs[:], sr[:])
                for f in range(FC):
                    ho = t_new([P, HW], F32, "hio", 3)
                    nc.sync.dma_start(ho[:], hd[f][:, hs])
                    ot = t_new([P, HW], F32, "otile", 1)
                    nc.vector.tensor_mul(ot[:], ho[:], rs[:])
                    nc.vector.tensor_scalar_mul(ot[:], ot[:], nw_sb[:, f:f + 1])
                    nc.sync.dma_start(out[f * P:(f + 1) * P, hs], ot[:])

    nc.compile()
    return nc


# ---------------------------------------------------------------------------
# host-side input prep
# ---------------------------------------------------------------------------
def prep_inputs(x, proj_w, Wq, Wk, Wv, Wo, qn, kn, ln1, ln2, Wg, Wu, Wd, norm_w,
                n_layers=L):
    bf = ml_dtypes.bfloat16
    f32 = np.float32

    # rope tables (positions 0..S-1)
    inv = 1.0 / (THETA ** (np.arange(0, D, 2, dtype=np.float64) / D))
    frq = np.arange(S, dtype=np.float64)[:, None] * inv[None, :]        # [S, D/2]
    emb = np.concatenate([frq, frq], axis=-1)                            # [S, D]
    cosT = np.cos(emb).T.astype(f32)                                     # [D, S]
    sinT = np.sin(emb).T.astype(f32)
    sign = np.ones((D, 1), f32)
    sign[:D // 2] = -1.0
    sc = D ** -0.5

    qn = np.asarray(qn, f32)[:n_layers]
    kn = np.asarray(kn, f32)[:n_layers]
    cq = np.stack([cosT * qn[l][:, None] * sc for l in range(n_layers)]).astype(bf)
    sq = np.stack([sinT * np.roll(qn[l], D // 2)[:, None] * sign * sc
                   for l in range(n_layers)]).astype(bf)
    ck = np.stack([cosT * kn[l][:, None] for l in range(n_layers)]).astype(bf)
    sk = np.stack([sinT * np.roll(kn[l], D // 2)[:, None] * sign
                   for l in range(n_layers)]).astype(bf)

    idx = np.arange(P)
    mge = (idx[:, None] >= idx[None, :]).astype(bf)
    mle = (idx[:, None] <= idx[None, :]).astype(bf)

    x = np.asarray(x, f32)
    wp = np.ascontiguousarray(np.asarray(proj_w, f32).T).astype(bf)      # [TIN, HID]
    nwc = np.asarray(norm_w, f32).reshape(HID, 1)

    in_maps = []
    for core in range(8):
        b = core // TP
        r = core % TP
        m = {
            "xb": np.ascontiguousarray(x[b].T).astype(bf),               # [TIN, S]
            "wp": wp,
            "cq": cq, "sq": sq, "ck": ck, "sk": sk,
            "nw": nwc, "mge": mge, "mle": mle,
        }
        wq_l, wk_l, wv_l, wo_l, wg_l, wu_l, wd_l = [], [], [], [], [], [], []
        for l in range(n_layers):
            l1 = np.asarray(ln1[l], f32)[None, :]
            l2 = np.asarray(ln2[l], f32)[None, :]
            wq_l.append((np.asarray(Wq[l], f32) * l1).T[:, r * NQ * D:(r + 1) * NQ * D])
            wk_l.append((np.asarray(Wk[l], f32) * l1).T[:, r * NKV * D:(r + 1) * NKV * D])
            wv_l.append((np.asarray(Wv[l], f32) * l1).T[:, r * NKV * D:(r + 1) * NKV * D])
            wo_l.append(np.asarray(Wo[l], f32).T[r * NQ * D:(r + 1) * NQ * D, :])
            wg_l.append((np.asarray(Wg[l], f32) * l2).T[:, r * IS:(r + 1) * IS])
            wu_l.append((np.asarray(Wu[l], f32) * l2).T[:, r * IS:(r + 1) * IS])
            wd_l.append(np.asarray(Wd[l], f32).T[r * IS:(r + 1) * IS, :])
        m["wq"] = np.ascontiguousarray(np.stack(wq_l)).astype(bf)
        m["wk"] = np.ascontiguousarray(np.stack(wk_l)).astype(bf)
        m["wv"] = np.ascontiguousarray(np.stack(wv_l)).astype(bf)
        m["wo"] = np.ascontiguousarray(np.stack(wo_l)).astype(bf)
        m["wg"] = np.ascontiguousarray(np.stack(wg_l)).astype(bf)
        m["wu"] = np.ascontiguousarray(np.stack(wu_l)).astype(bf)
        m["wd"] = np.ascontiguousarray(np.stack(wd_l)).astype(bf)
        in_maps.append(m)
    return in_maps


_NC_CACHE = {}


def get_program(n_layers=L, repeat=1, ar_f32=False, no_coll=False):
    key = (n_layers, repeat, ar_f32, no_coll)
    if key not in _NC_CACHE:
        _NC_CACHE[key] = build_program(n_layers, repeat, ar_f32, no_coll)
    return _NC_CACHE[key]


def kernel(**inputs) -> np.ndarray:
    nc = get_program()
    in_maps = prep_inputs(**inputs)
    res = run_bass_kernel_spmd(nc, in_maps, list(range(8))).results
    y = np.empty((B, S, HID), np.float32)
    for b in range(B):
        y[b] = res[b * TP]["out"].T
    return y
